# revision 36
# baseline (speedup 1.0000x reference)
"""AnchorTarget (faster-rcnn anchor target layer) on 8 TRN2 NeuronCores.

Strategy: shard the anchor axis (A = 128*128*9 = 147456) across 8 cores
(18432 anchors/core). Each core processes 144 tiles of 128 anchors x 256 gts:
  - IoU tile [128a, 256g] via tensor_scalar / scalar_tensor_tensor ops
    (anchor coords are per-partition scalars, gt coords are replicated rows)
  - per-anchor max/argmax over g via DVE max/max_index (top-8 ops)
  - bbox regression targets via gpsimd indirect_copy gather of the argmax gt
  - per-gt max/argmax over anchors via PE transposes into PSUM windows
    (4 tiles = 512 anchors per window), then DVE max/max_index + running merge
Host merges the 8 per-core per-gt argmax candidates (a 256-long argmax) and
applies the keras-rcnn scatter-add label fixup at the <=256 winning anchors.
"""
import os
import sys

import numpy as np

if "/opt/trn_rl_repo" not in sys.path:
    sys.path.insert(0, "/opt/trn_rl_repo")

import concourse.bacc as bacc
import concourse.bass as bass
import concourse.mybir as mybir
from concourse.bass_utils import run_bass_kernel_spmd
from concourse.tile import TileContext

F32 = mybir.dt.float32
U16 = mybir.dt.uint16

N_CORES = 8
RR = CC = 128
STRIDE = 16
NA = 9
A = RR * CC * NA            # 147456
G = 256
APC = A // N_CORES          # 18432 anchors per core
P = 128                     # partitions
T = APC // P                # 144 anchor tiles per core
WIN = 4                     # anchor tiles per per-gt reduction window
NWIN = T // WIN             # 36

IN_SPECS = [
    ("ax1", [P, T]), ("ay1", [P, T]), ("ax2p", [P, T]), ("ay2p", [P, T]),
    ("areaa", [P, T]), ("ins01", [P, T]),
    ("e4", [P, 4 * T]), ("ewh4", [P, 4 * T]), ("ins4", [P, 4 * T]),
    ("gx1", [P, G]), ("gy1", [P, G]), ("gx2p", [P, G]), ("gy2p", [P, G]),
    ("areag", [P, G]),
    ("gt4p", [P, 8]), ("iota", [P, G]), ("ident", [P, P]),
]


def _mkanchors(ws, hs, cx, cy):
    return np.stack([cx - 0.5 * (ws - 1), cy - 0.5 * (hs - 1),
                     cx + 0.5 * (ws - 1), cy + 0.5 * (hs - 1)], axis=1)


def _base_anchors(base_size=16, ratios=(0.5, 1.0, 2.0), scales=(8, 16, 32)):
    ratios = np.asarray(ratios, dtype=np.float64)
    scales = np.asarray(scales, dtype=np.float64)
    w = h = float(base_size)
    cx = cy = 0.5 * (base_size - 1)
    size = w * h
    ws = np.round(np.sqrt(size / ratios))
    hs = np.round(ws * ratios)
    ratio_anchors = _mkanchors(ws, hs, cx, cy)
    out = []
    for x1, y1, x2, y2 in ratio_anchors:
        aw = x2 - x1 + 1
        ah = y2 - y1 + 1
        acx = x1 + 0.5 * (aw - 1)
        acy = y1 + 0.5 * (ah - 1)
        out.append(_mkanchors(aw * scales, ah * scales, acx, acy))
    return np.concatenate(out, axis=0).astype(np.float32)


def _anchors_np():
    base = _base_anchors()
    sx = np.arange(RR) * STRIDE
    sy = np.arange(CC) * STRIDE
    SX, SY = np.meshgrid(sx, sy)
    shifts = np.stack([SX.ravel(), SY.ravel(), SX.ravel(), SY.ravel()],
                      axis=1).astype(np.float32)
    return (shifts[:, None, :] + base[None, :, :]).reshape(-1, 4)


def _build_graph():
    nc = bacc.Bacc("TRN2", target_bir_lowering=False, debug=False,
                   num_devices=N_CORES)
    AL = mybir.AluOpType
    AF = mybir.ActivationFunctionType

    ins = {}
    for name, shape in IN_SPECS:
        ins[name] = nc.declare_dram_parameter(name, shape, F32, isOutput=False)
    out_labels = nc.declare_dram_parameter("labels", [P, T], F32, isOutput=True)
    out_bbox = nc.declare_dram_parameter("bbox", [P, 4 * T], F32, isOutput=True)
    out_gtmax = nc.declare_dram_parameter("gtmax", [P, 2], F32, isOutput=True)
    out_gtidx = nc.declare_dram_parameter("gtidx", [P, 2], F32, isOutput=True)

    with TileContext(nc) as tc:
        with tc.tile_pool(name="cpool", bufs=1) as cpool, \
             tc.tile_pool(name="wpool", bufs=3) as wpool, \
             tc.tile_pool(name="apool", bufs=1) as apool, \
             tc.tile_pool(name="ppool", bufs=2, space="PSUM") as ppool:

            C = {}
            for name, shape in IN_SPECS:
                tl = cpool.tile(shape, F32, name=f"c_{name}")
                nc.sync.dma_start(out=tl[:, :], in_=ins[name][:, :])
                C[name] = tl

            mo_all = apool.tile([P, T], F32, name="mo_all")
            labels_sb = apool.tile([P, T], F32, name="labels_sb")
            bbox_sb = apool.tile([P, 4 * T], F32, name="bbox_sb")
            runmax = apool.tile([P, 2], F32, name="runmax")
            runidx = apool.tile([P, 2], F32, name="runidx")
            nc.vector.memset(runmax[:, :], -1.0)
            nc.vector.memset(runidx[:, :], 0.0)

            for w in range(NWIN):
                psA = ppool.tile([P, P * WIN], F32, name="psA", tag="psA")
                psB = ppool.tile([P, P * WIN], F32, name="psB", tag="psB")
                g4ps = ppool.tile([P, 4 * WIN], F32, name="g4ps", tag="g4ps")
                for k in range(WIN):
                    t = w * WIN + k
                    sax1 = C["ax1"][:, t:t + 1]
                    say1 = C["ay1"][:, t:t + 1]
                    sax2p = C["ax2p"][:, t:t + 1]
                    say2p = C["ay2p"][:, t:t + 1]
                    sareaa = C["areaa"][:, t:t + 1]

                    xlo = wpool.tile([P, G], F32, name="xlo")
                    nc.vector.tensor_scalar(out=xlo[:, :], in0=C["gx1"][:, :],
                                            scalar1=sax1, scalar2=None,
                                            op0=AL.max)
                    ylo = wpool.tile([P, G], F32, name="ylo")
                    nc.vector.tensor_scalar(out=ylo[:, :], in0=C["gy1"][:, :],
                                            scalar1=say1, scalar2=None,
                                            op0=AL.max)
                    iw = wpool.tile([P, G], F32, name="iw")
                    nc.vector.scalar_tensor_tensor(out=iw[:, :],
                                                   in0=C["gx2p"][:, :],
                                                   scalar=sax2p, in1=xlo[:, :],
                                                   op0=AL.min, op1=AL.subtract)
                    ih = wpool.tile([P, G], F32, name="ih")
                    nc.vector.scalar_tensor_tensor(out=ih[:, :],
                                                   in0=C["gy2p"][:, :],
                                                   scalar=say2p, in1=ylo[:, :],
                                                   op0=AL.min, op1=AL.subtract)
                    ihr = wpool.tile([P, G], F32, name="ihr")
                    nc.scalar.activation(out=ihr[:, :], in_=ih[:, :],
                                         func=AF.Relu)
                    # inter = relu(iw) * ihr, relu fused into the STT
                    inter = wpool.tile([P, G], F32, name="inter")
                    nc.vector.scalar_tensor_tensor(out=inter[:, :],
                                                   in0=iw[:, :], scalar=0.0,
                                                   in1=ihr[:, :],
                                                   op0=AL.max, op1=AL.mult)
                    union = wpool.tile([P, G], F32, name="union")
                    nc.vector.scalar_tensor_tensor(out=union[:, :],
                                                   in0=C["areag"][:, :],
                                                   scalar=sareaa,
                                                   in1=inter[:, :],
                                                   op0=AL.add, op1=AL.subtract)
                    runion = wpool.tile([P, G], F32, name="runion")
                    nc.vector.reciprocal(out=runion[:, :], in_=union[:, :])
                    iou = wpool.tile([P, G], F32, name="iou")
                    nc.vector.tensor_tensor(out=iou[:, :], in0=inter[:, :],
                                            in1=runion[:, :], op=AL.mult)
                    nc.tensor.transpose(psA[:, k * P:(k + 1) * P],
                                        iou[:, 0:P], C["ident"][:, :])
                    nc.tensor.transpose(psB[:, k * P:(k + 1) * P],
                                        iou[:, P:G], C["ident"][:, :])

                    mx8 = wpool.tile([P, 8], F32, name="mx8")
                    nc.vector.max(out=mx8[:, :], in_=iou[:, :])
                    ix8 = wpool.tile([P, 8], U16, name="ix8")
                    nc.vector.max_index(out=ix8[:, :], in_max=mx8[:, :],
                                        in_values=iou[:, :])
                    nc.scalar.copy(out=mo_all[:, t:t + 1], in_=mx8[:, 0:1])
                    idxf = wpool.tile([P, 1], F32, name="idxf")
                    nc.scalar.copy(out=idxf[:, :], in_=ix8[:, 0:1])
                    onehot = wpool.tile([P, G], F32, name="onehot")
                    nc.vector.tensor_scalar(out=onehot[:, :],
                                            in0=C["iota"][:, :],
                                            scalar1=idxf[:, 0:1], scalar2=None,
                                            op0=AL.is_equal)
                    psOH = ppool.tile([P, G], F32, name="psOH", tag="psOH")
                    nc.tensor.transpose(psOH[:, 0:P], onehot[:, 0:P],
                                        C["ident"][:, :])
                    nc.tensor.transpose(psOH[:, P:G], onehot[:, P:G],
                                        C["ident"][:, :])
                    ohT = wpool.tile([P, G], F32, name="ohT")
                    nc.scalar.copy(out=ohT[:, :], in_=psOH[:, :])
                    nc.tensor.matmul(g4ps[:, 4 * k:4 * k + 4], ohT[:, 0:P],
                                     C["gt4p"][:, 0:4], start=True, stop=False)
                    nc.tensor.matmul(g4ps[:, 4 * k:4 * k + 4], ohT[:, P:G],
                                     C["gt4p"][:, 4:8], start=False, stop=True)

                # batched bbox epilogue for the window's 4 tiles
                cs = slice(4 * WIN * w, 4 * WIN * (w + 1))
                d16 = wpool.tile([P, 4 * WIN], F32, name="d16")
                nc.vector.tensor_tensor(out=d16[:, :], in0=g4ps[:, :],
                                        in1=C["e4"][:, cs], op=AL.subtract)
                bb16 = wpool.tile([P, 4 * WIN], F32, name="bb16")
                nc.vector.tensor_tensor(out=bb16[:, :], in0=d16[:, :],
                                        in1=C["ewh4"][:, cs], op=AL.mult)
                for k in range(WIN):
                    c0 = 4 * (WIN * w + k) + 2
                    nc.scalar.activation(out=bb16[:, 4 * k + 2:4 * k + 4],
                                         in_=bb16[:, 4 * k + 2:4 * k + 4],
                                         func=AF.Ln)
                nc.vector.tensor_tensor(out=bbox_sb[:, cs], in0=bb16[:, :],
                                        in1=C["ins4"][:, cs], op=AL.mult)

                # per-gt reduction over this window of 512 anchors
                for ch, ps in ((0, psA), (1, psB)):
                    wbuf = wpool.tile([P, P * WIN], F32, name=f"wbuf{ch}")
                    nc.scalar.copy(out=wbuf[:, :], in_=ps[:, :])
                    wm8 = wpool.tile([P, 8], F32, name=f"wm8{ch}")
                    nc.vector.max(out=wm8[:, :], in_=wbuf[:, :])
                    wi8 = wpool.tile([P, 8], U16, name=f"wi8{ch}")
                    nc.vector.max_index(out=wi8[:, :], in_max=wm8[:, :],
                                        in_values=wbuf[:, :])
                    better = wpool.tile([P, 1], F32, name=f"better{ch}")
                    nc.vector.tensor_tensor(out=better[:, :], in0=wm8[:, 0:1],
                                            in1=runmax[:, ch:ch + 1],
                                            op=AL.is_gt)
                    wif = wpool.tile([P, 1], F32, name=f"wif{ch}")
                    nc.scalar.copy(out=wif[:, :], in_=wi8[:, 0:1])
                    dI = wpool.tile([P, 1], F32, name=f"dI{ch}")
                    nc.vector.scalar_tensor_tensor(
                        out=dI[:, :], in0=wif[:, :],
                        scalar=float(w * WIN * P),
                        in1=runidx[:, ch:ch + 1],
                        op0=AL.add, op1=AL.subtract)
                    nc.vector.scalar_tensor_tensor(
                        out=runidx[:, ch:ch + 1], in0=dI[:, :],
                        scalar=better[:, 0:1],
                        in1=runidx[:, ch:ch + 1],
                        op0=AL.mult, op1=AL.add)
                    nc.vector.tensor_tensor(out=runmax[:, ch:ch + 1],
                                            in0=runmax[:, ch:ch + 1],
                                            in1=wm8[:, 0:1], op=AL.max)

            # labels (batched): -1 + (mo<0.3) + 2*(mo>=0.7), then inside mask
            lt03 = apool.tile([P, T], F32, name="lt03")
            nc.vector.tensor_scalar(out=lt03[:, :], in0=mo_all[:, :],
                                    scalar1=0.3, scalar2=None, op0=AL.is_lt)
            ge07 = apool.tile([P, T], F32, name="ge07")
            nc.vector.tensor_scalar(out=ge07[:, :], in0=mo_all[:, :],
                                    scalar1=0.7, scalar2=None, op0=AL.is_ge)
            l1 = apool.tile([P, T], F32, name="l1")
            nc.vector.scalar_tensor_tensor(out=l1[:, :], in0=ge07[:, :],
                                           scalar=2.0, in1=lt03[:, :],
                                           op0=AL.mult, op1=AL.add)
            lm = apool.tile([P, T], F32, name="lm")
            nc.vector.tensor_tensor(out=lm[:, :], in0=l1[:, :],
                                    in1=C["ins01"][:, :], op=AL.mult)
            nc.vector.tensor_scalar(out=labels_sb[:, :], in0=lm[:, :],
                                    scalar1=-1.0, scalar2=None, op0=AL.add)

            nc.sync.dma_start(out=out_labels[:, :], in_=labels_sb[:, :])
            nc.sync.dma_start(out=out_bbox[:, :], in_=bbox_sb[:, :])
            nc.sync.dma_start(out=out_gtmax[:, :], in_=runmax[:, :])
            nc.sync.dma_start(out=out_gtidx[:, :], in_=runidx[:, :])
    nc.finalize()
    return nc


_GRAPH = None
LAST_RESULT = None


def _get_graph():
    global _GRAPH
    if _GRAPH is None:
        _GRAPH = _build_graph()
    return _GRAPH


def prepare_in_maps(inputs):
    in_maps, _ = _prepare(inputs["gt_boxes"], inputs["metadata"])
    return in_maps


def _prepare(gt_boxes, metadata):
    f1 = np.float32(1.0)
    half = np.float32(0.5)
    gt = np.asarray(gt_boxes, dtype=np.float32)[0]
    meta = np.asarray(metadata, dtype=np.float32)[0]
    H, W = np.float32(meta[0]), np.float32(meta[1])

    anchors = _anchors_np()
    x1, y1, x2, y2 = anchors[:, 0], anchors[:, 1], anchors[:, 2], anchors[:, 3]
    inside = (x1 >= 0) & (y1 >= 0) & (x2 < W) & (y2 < H)
    ew = x2 - x1 + f1
    eh = y2 - y1 + f1
    ecx = x1 + half * ew
    ecy = y1 + half * eh
    areaa = ew * eh

    gw = gt[:, 2] - gt[:, 0] + f1
    gh = gt[:, 3] - gt[:, 1] + f1
    gcx = gt[:, 0] + half * gw
    gcy = gt[:, 1] + half * gh
    areag = gw * gh
    gtm4 = np.stack([gcx, gcy, gw, gh], axis=1)

    def rep(v):
        return np.ascontiguousarray(
            np.broadcast_to(v[None, :].astype(np.float32), (P, G)))

    base = {
        "gx1": rep(gt[:, 0]), "gy1": rep(gt[:, 1]),
        "gx2p": rep(gt[:, 2] + f1), "gy2p": rep(gt[:, 3] + f1),
        "areag": rep(areag),
        "gt4p": np.ascontiguousarray(np.concatenate(
            [gtm4[0:P], gtm4[P:G]], axis=1).astype(np.float32)),
        "iota": np.ascontiguousarray(np.broadcast_to(
            np.arange(G, dtype=np.float32)[None, :], (P, G))),
        "ident": np.eye(P, dtype=np.float32),
    }

    zeros = np.zeros(APC, np.float32)
    in_maps = []
    for c in range(N_CORES):
        sl = slice(c * APC, (c + 1) * APC)

        def pt(v):
            # t-major within-core layout: [p, t] holds anchor t*128 + p
            return np.ascontiguousarray(
                v[sl].reshape(T, P).T.astype(np.float32))

        im = dict(base)
        im["ax1"] = pt(x1)
        im["ay1"] = pt(y1)
        im["ax2p"] = pt(x2 + f1)
        im["ay2p"] = pt(y2 + f1)
        # outside anchors get a huge area => iou ~ 1e-5, never wins a per-gt
        # argmax (real per-gt maxima are >= ~0.015) and maps to label 0
        # pre-unmap, exactly as the reference's masked/unmapped semantics need
        im["areaa"] = pt(areaa + (~inside).astype(np.float32) * np.float32(1e9))
        im["ins01"] = pt(inside.astype(np.float32))
        e4 = np.stack([ecx[sl], ecy[sl], zeros, zeros], axis=1)
        im["e4"] = np.ascontiguousarray(
            e4.reshape(T, P, 4).transpose(1, 0, 2).reshape(P, 4 * T)
            .astype(np.float32))
        iew = np.float32(1.0) / ew[sl]
        ieh = np.float32(1.0) / eh[sl]
        ewh = np.stack([iew, ieh, iew, ieh], axis=1)
        im["ewh4"] = np.ascontiguousarray(
            ewh.reshape(T, P, 4).transpose(1, 0, 2).reshape(P, 4 * T)
            .astype(np.float32))
        insf = inside[sl].astype(np.float32)
        ins4 = np.stack([insf] * 4, axis=1)
        im["ins4"] = np.ascontiguousarray(
            ins4.reshape(T, P, 4).transpose(1, 0, 2).reshape(P, 4 * T)
            .astype(np.float32))
        in_maps.append(im)
    return in_maps, inside


def kernel(scores, gt_boxes, metadata):
    global LAST_RESULT
    in_maps, inside = _prepare(gt_boxes, metadata)
    nc = _get_graph()
    trace = bool(int(os.environ.get("BASSK_TRACE", "0")))
    res = run_bass_kernel_spmd(nc, in_maps, core_ids=list(range(N_CORES)),
                               trace=trace)
    LAST_RESULT = res
    results = res.results

    labels = np.empty(A, np.float32)
    bbox = np.empty((A, 4), np.float32)
    Mv = np.empty((N_CORES, G), np.float32)
    Iv = np.empty((N_CORES, G), np.int64)
    for c, r in enumerate(results):
        # [p, t] holds anchor t*128+p of this core's shard
        labels[c * APC:(c + 1) * APC] = np.asarray(r["labels"]).T.reshape(-1)
        bbox[c * APC:(c + 1) * APC] = (
            np.asarray(r["bbox"]).reshape(P, T, 4).transpose(1, 0, 2)
            .reshape(APC, 4))
        gm = np.asarray(r["gtmax"])
        gi = np.asarray(r["gtidx"])
        for ch in (0, 1):
            g = ch * P + np.arange(P)
            Mv[c, g] = gm[:, ch]
            # window position w*512 + j IS the within-core anchor index
            Iv[c, g] = c * APC + gi[:, ch].astype(np.int64)

    cstar = np.argmax(Mv, axis=0)
    gt_argmax = Iv[cstar, np.arange(G)]
    counts = np.bincount(gt_argmax, minlength=A)
    for a in np.nonzero(counts)[0]:
        if not inside[a]:
            continue
        k = counts[a]
        l = labels[a]
        if l == 0.0:
            labels[a] = np.float32(k)
        elif l == -1.0:
            labels[a] = np.float32(2 * k - 1)
    return labels[None, :], bbox[None, :, :]


# revision 37
# speedup vs baseline: 1.0158x; 1.0158x over previous
"""AnchorTarget (faster-rcnn anchor target layer) on 8 TRN2 NeuronCores.

Strategy: shard the anchor axis (A = 128*128*9 = 147456) across 8 cores
(18432 anchors/core). Each core processes 144 tiles of 128 anchors x 256 gts:
  - IoU tile [128a, 256g] via tensor_scalar / scalar_tensor_tensor ops
    (anchor coords are per-partition scalars, gt coords are replicated rows)
  - per-anchor max/argmax over g via DVE max/max_index (top-8 ops)
  - bbox regression targets via gpsimd indirect_copy gather of the argmax gt
  - per-gt max/argmax over anchors via PE transposes into PSUM windows
    (4 tiles = 512 anchors per window), then DVE max/max_index + running merge
Host merges the 8 per-core per-gt argmax candidates (a 256-long argmax) and
applies the keras-rcnn scatter-add label fixup at the <=256 winning anchors.
"""
import os
import sys

import numpy as np

if "/opt/trn_rl_repo" not in sys.path:
    sys.path.insert(0, "/opt/trn_rl_repo")

import concourse.bacc as bacc
import concourse.bass as bass
import concourse.mybir as mybir
from concourse.bass_utils import run_bass_kernel_spmd
from concourse.tile import TileContext

F32 = mybir.dt.float32
U16 = mybir.dt.uint16

N_CORES = 8
RR = CC = 128
STRIDE = 16
NA = 9
A = RR * CC * NA            # 147456
G = 256
APC = A // N_CORES          # 18432 anchors per core
P = 128                     # partitions
T = APC // P                # 144 anchor tiles per core
WIN = 4                     # anchor tiles per per-gt reduction window
NWIN = T // WIN             # 36

IN_SPECS = [
    ("ax1", [P, T]), ("ay1", [P, T]), ("ax2p", [P, T]), ("ay2p", [P, T]),
    ("areaa", [P, T]), ("ins01", [P, T]),
    ("e4", [P, 4 * T]), ("ewh4", [P, 4 * T]), ("ins4", [P, 4 * T]),
    ("gx1", [P, G]), ("gy1", [P, G]), ("gx2p", [P, G]), ("gy2p", [P, G]),
    ("areag", [P, G]),
    ("gt4p", [P, 8]), ("iota", [P, G]), ("ident", [P, P]),
]


def _mkanchors(ws, hs, cx, cy):
    return np.stack([cx - 0.5 * (ws - 1), cy - 0.5 * (hs - 1),
                     cx + 0.5 * (ws - 1), cy + 0.5 * (hs - 1)], axis=1)


def _base_anchors(base_size=16, ratios=(0.5, 1.0, 2.0), scales=(8, 16, 32)):
    ratios = np.asarray(ratios, dtype=np.float64)
    scales = np.asarray(scales, dtype=np.float64)
    w = h = float(base_size)
    cx = cy = 0.5 * (base_size - 1)
    size = w * h
    ws = np.round(np.sqrt(size / ratios))
    hs = np.round(ws * ratios)
    ratio_anchors = _mkanchors(ws, hs, cx, cy)
    out = []
    for x1, y1, x2, y2 in ratio_anchors:
        aw = x2 - x1 + 1
        ah = y2 - y1 + 1
        acx = x1 + 0.5 * (aw - 1)
        acy = y1 + 0.5 * (ah - 1)
        out.append(_mkanchors(aw * scales, ah * scales, acx, acy))
    return np.concatenate(out, axis=0).astype(np.float32)


def _anchors_np():
    base = _base_anchors()
    sx = np.arange(RR) * STRIDE
    sy = np.arange(CC) * STRIDE
    SX, SY = np.meshgrid(sx, sy)
    shifts = np.stack([SX.ravel(), SY.ravel(), SX.ravel(), SY.ravel()],
                      axis=1).astype(np.float32)
    return (shifts[:, None, :] + base[None, :, :]).reshape(-1, 4)


def _build_graph():
    nc = bacc.Bacc("TRN2", target_bir_lowering=False, debug=False,
                   num_devices=N_CORES)
    AL = mybir.AluOpType
    AF = mybir.ActivationFunctionType

    ins = {}
    for name, shape in IN_SPECS:
        ins[name] = nc.declare_dram_parameter(name, shape, F32, isOutput=False)
    out_labels = nc.declare_dram_parameter("labels", [P, T], F32, isOutput=True)
    out_bbox = nc.declare_dram_parameter("bbox", [P, 4 * T], F32, isOutput=True)
    out_gtmax = nc.declare_dram_parameter("gtmax", [P, 2], F32, isOutput=True)
    out_gtidx = nc.declare_dram_parameter("gtidx", [P, 2], F32, isOutput=True)

    with TileContext(nc) as tc:
        with tc.tile_pool(name="cpool", bufs=1) as cpool, \
             tc.tile_pool(name="wpool", bufs=3) as wpool, \
             tc.tile_pool(name="apool", bufs=1) as apool, \
             tc.tile_pool(name="ppool", bufs=2, space="PSUM") as ppool:

            C = {}
            for name, shape in IN_SPECS:
                tl = cpool.tile(shape, F32, name=f"c_{name}")
                nc.sync.dma_start(out=tl[:, :], in_=ins[name][:, :])
                C[name] = tl

            mo_all = apool.tile([P, T], F32, name="mo_all")
            labels_sb = apool.tile([P, T], F32, name="labels_sb")
            bbox_sb = apool.tile([P, 4 * T], F32, name="bbox_sb")
            runmax = apool.tile([P, 2], F32, name="runmax")
            runidx = apool.tile([P, 2], F32, name="runidx")
            nc.vector.memset(runmax[:, :], -1.0)
            nc.vector.memset(runidx[:, :], 0.0)

            for w in range(NWIN):
                psA = ppool.tile([P, P * WIN], F32, name="psA", tag="psA")
                psB = ppool.tile([P, P * WIN], F32, name="psB", tag="psB")
                g4ps = ppool.tile([P, 4 * WIN], F32, name="g4ps", tag="g4ps")
                for k in range(WIN):
                    t = w * WIN + k
                    sax1 = C["ax1"][:, t:t + 1]
                    say1 = C["ay1"][:, t:t + 1]
                    sax2p = C["ax2p"][:, t:t + 1]
                    say2p = C["ay2p"][:, t:t + 1]
                    sareaa = C["areaa"][:, t:t + 1]

                    xlo = wpool.tile([P, G], F32, name="xlo")
                    nc.vector.tensor_scalar(out=xlo[:, :], in0=C["gx1"][:, :],
                                            scalar1=sax1, scalar2=None,
                                            op0=AL.max)
                    ylo = wpool.tile([P, G], F32, name="ylo")
                    nc.vector.tensor_scalar(out=ylo[:, :], in0=C["gy1"][:, :],
                                            scalar1=say1, scalar2=None,
                                            op0=AL.max)
                    iw = wpool.tile([P, G], F32, name="iw")
                    nc.vector.scalar_tensor_tensor(out=iw[:, :],
                                                   in0=C["gx2p"][:, :],
                                                   scalar=sax2p, in1=xlo[:, :],
                                                   op0=AL.min, op1=AL.subtract)
                    ih = wpool.tile([P, G], F32, name="ih")
                    nc.vector.scalar_tensor_tensor(out=ih[:, :],
                                                   in0=C["gy2p"][:, :],
                                                   scalar=say2p, in1=ylo[:, :],
                                                   op0=AL.min, op1=AL.subtract)
                    ihr = wpool.tile([P, G], F32, name="ihr")
                    nc.scalar.activation(out=ihr[:, :], in_=ih[:, :],
                                         func=AF.Relu)
                    # inter = relu(iw) * ihr, relu fused into the STT
                    inter = wpool.tile([P, G], F32, name="inter")
                    nc.vector.scalar_tensor_tensor(out=inter[:, :],
                                                   in0=iw[:, :], scalar=0.0,
                                                   in1=ihr[:, :],
                                                   op0=AL.max, op1=AL.mult)
                    union = wpool.tile([P, G], F32, name="union")
                    nc.vector.scalar_tensor_tensor(out=union[:, :],
                                                   in0=C["areag"][:, :],
                                                   scalar=sareaa,
                                                   in1=inter[:, :],
                                                   op0=AL.add, op1=AL.subtract)
                    runion = wpool.tile([P, G], F32, name="runion")
                    nc.vector.reciprocal(out=runion[:, :], in_=union[:, :])
                    iou = wpool.tile([P, G], F32, name="iou")
                    nc.vector.tensor_tensor(out=iou[:, :], in0=inter[:, :],
                                            in1=runion[:, :], op=AL.mult)
                    nc.tensor.transpose(psA[:, k * P:(k + 1) * P],
                                        iou[:, 0:P], C["ident"][:, :])
                    nc.tensor.transpose(psB[:, k * P:(k + 1) * P],
                                        iou[:, P:G], C["ident"][:, :])

                    mx8 = wpool.tile([P, 8], F32, name="mx8")
                    nc.vector.max(out=mx8[:, :], in_=iou[:, :])
                    ix8 = wpool.tile([P, 8], U16, name="ix8")
                    nc.vector.max_index(out=ix8[:, :], in_max=mx8[:, :],
                                        in_values=iou[:, :])
                    nc.scalar.copy(out=mo_all[:, t:t + 1], in_=mx8[:, 0:1])
                    idxf = wpool.tile([P, 1], F32, name="idxf")
                    nc.scalar.copy(out=idxf[:, :], in_=ix8[:, 0:1])
                    onehot = wpool.tile([P, G], F32, name="onehot")
                    nc.vector.tensor_scalar(out=onehot[:, :],
                                            in0=C["iota"][:, :],
                                            scalar1=idxf[:, 0:1], scalar2=None,
                                            op0=AL.is_equal)
                    psOH = ppool.tile([P, G], F32, name="psOH", tag="psOH")
                    nc.tensor.transpose(psOH[:, 0:P], onehot[:, 0:P],
                                        C["ident"][:, :])
                    nc.tensor.transpose(psOH[:, P:G], onehot[:, P:G],
                                        C["ident"][:, :])
                    ohT = wpool.tile([P, G], F32, name="ohT")
                    nc.scalar.copy(out=ohT[:, :], in_=psOH[:, :])
                    nc.tensor.matmul(g4ps[:, 4 * k:4 * k + 4], ohT[:, 0:P],
                                     C["gt4p"][:, 0:4], start=True, stop=False)
                    nc.tensor.matmul(g4ps[:, 4 * k:4 * k + 4], ohT[:, P:G],
                                     C["gt4p"][:, 4:8], start=False, stop=True)

                # batched bbox epilogue for the window's 4 tiles
                cs = slice(4 * WIN * w, 4 * WIN * (w + 1))
                d16 = wpool.tile([P, 4 * WIN], F32, name="d16")
                nc.vector.tensor_tensor(out=d16[:, :], in0=g4ps[:, :],
                                        in1=C["e4"][:, cs], op=AL.subtract)
                bb16 = wpool.tile([P, 4 * WIN], F32, name="bb16")
                nc.vector.tensor_tensor(out=bb16[:, :], in0=d16[:, :],
                                        in1=C["ewh4"][:, cs], op=AL.mult)
                for k in range(WIN):
                    nc.scalar.activation(out=bb16[:, 4 * k + 2:4 * k + 4],
                                         in_=bb16[:, 4 * k + 2:4 * k + 4],
                                         func=AF.Ln)
                nc.vector.tensor_tensor(out=bbox_sb[:, cs], in0=bb16[:, :],
                                        in1=C["ins4"][:, cs], op=AL.mult)

                # per-gt reduction over this window of 512 anchors
                for ch, ps in ((0, psA), (1, psB)):
                    wbuf = wpool.tile([P, P * WIN], F32, name=f"wbuf{ch}")
                    nc.scalar.copy(out=wbuf[:, :], in_=ps[:, :])
                    wm8 = wpool.tile([P, 8], F32, name=f"wm8{ch}")
                    nc.vector.max(out=wm8[:, :], in_=wbuf[:, :])
                    wi8 = wpool.tile([P, 8], U16, name=f"wi8{ch}")
                    nc.vector.max_index(out=wi8[:, :], in_max=wm8[:, :],
                                        in_values=wbuf[:, :])
                    better = wpool.tile([P, 1], F32, name=f"better{ch}")
                    nc.vector.tensor_tensor(out=better[:, :], in0=wm8[:, 0:1],
                                            in1=runmax[:, ch:ch + 1],
                                            op=AL.is_gt)
                    wif = wpool.tile([P, 1], F32, name=f"wif{ch}")
                    nc.scalar.copy(out=wif[:, :], in_=wi8[:, 0:1])
                    dI = wpool.tile([P, 1], F32, name=f"dI{ch}")
                    nc.vector.scalar_tensor_tensor(
                        out=dI[:, :], in0=wif[:, :],
                        scalar=float(w * WIN * P),
                        in1=runidx[:, ch:ch + 1],
                        op0=AL.add, op1=AL.subtract)
                    nc.vector.scalar_tensor_tensor(
                        out=runidx[:, ch:ch + 1], in0=dI[:, :],
                        scalar=better[:, 0:1],
                        in1=runidx[:, ch:ch + 1],
                        op0=AL.mult, op1=AL.add)
                    nc.vector.tensor_tensor(out=runmax[:, ch:ch + 1],
                                            in0=runmax[:, ch:ch + 1],
                                            in1=wm8[:, 0:1], op=AL.max)

            # labels (batched): -1 + (mo<0.3) + 2*(mo>=0.7), then inside mask
            lt03 = apool.tile([P, T], F32, name="lt03")
            nc.vector.tensor_scalar(out=lt03[:, :], in0=mo_all[:, :],
                                    scalar1=0.3, scalar2=None, op0=AL.is_lt)
            ge07 = apool.tile([P, T], F32, name="ge07")
            nc.vector.tensor_scalar(out=ge07[:, :], in0=mo_all[:, :],
                                    scalar1=0.7, scalar2=None, op0=AL.is_ge)
            l1 = apool.tile([P, T], F32, name="l1")
            nc.vector.scalar_tensor_tensor(out=l1[:, :], in0=ge07[:, :],
                                           scalar=2.0, in1=lt03[:, :],
                                           op0=AL.mult, op1=AL.add)
            lm = apool.tile([P, T], F32, name="lm")
            nc.vector.tensor_tensor(out=lm[:, :], in0=l1[:, :],
                                    in1=C["ins01"][:, :], op=AL.mult)
            nc.vector.tensor_scalar(out=labels_sb[:, :], in0=lm[:, :],
                                    scalar1=-1.0, scalar2=None, op0=AL.add)

            nc.sync.dma_start(out=out_labels[:, :], in_=labels_sb[:, :])
            nc.sync.dma_start(out=out_bbox[:, :], in_=bbox_sb[:, :])
            nc.sync.dma_start(out=out_gtmax[:, :], in_=runmax[:, :])
            nc.sync.dma_start(out=out_gtidx[:, :], in_=runidx[:, :])
    nc.finalize()
    return nc


_GRAPH = None
LAST_RESULT = None


def _get_graph():
    global _GRAPH
    if _GRAPH is None:
        _GRAPH = _build_graph()
    return _GRAPH


def prepare_in_maps(inputs):
    in_maps, _ = _prepare(inputs["gt_boxes"], inputs["metadata"])
    return in_maps


def _prepare(gt_boxes, metadata):
    f1 = np.float32(1.0)
    half = np.float32(0.5)
    gt = np.asarray(gt_boxes, dtype=np.float32)[0]
    meta = np.asarray(metadata, dtype=np.float32)[0]
    H, W = np.float32(meta[0]), np.float32(meta[1])

    anchors = _anchors_np()
    x1, y1, x2, y2 = anchors[:, 0], anchors[:, 1], anchors[:, 2], anchors[:, 3]
    inside = (x1 >= 0) & (y1 >= 0) & (x2 < W) & (y2 < H)
    ew = x2 - x1 + f1
    eh = y2 - y1 + f1
    ecx = x1 + half * ew
    ecy = y1 + half * eh
    areaa = ew * eh

    gw = gt[:, 2] - gt[:, 0] + f1
    gh = gt[:, 3] - gt[:, 1] + f1
    gcx = gt[:, 0] + half * gw
    gcy = gt[:, 1] + half * gh
    areag = gw * gh
    gtm4 = np.stack([gcx, gcy, gw, gh], axis=1)

    def rep(v):
        return np.ascontiguousarray(
            np.broadcast_to(v[None, :].astype(np.float32), (P, G)))

    base = {
        "gx1": rep(gt[:, 0]), "gy1": rep(gt[:, 1]),
        "gx2p": rep(gt[:, 2] + f1), "gy2p": rep(gt[:, 3] + f1),
        "areag": rep(areag),
        "gt4p": np.ascontiguousarray(np.concatenate(
            [gtm4[0:P], gtm4[P:G]], axis=1).astype(np.float32)),
        "iota": np.ascontiguousarray(np.broadcast_to(
            np.arange(G, dtype=np.float32)[None, :], (P, G))),
        "ident": np.eye(P, dtype=np.float32),
    }

    zeros = np.zeros(APC, np.float32)
    in_maps = []
    for c in range(N_CORES):
        sl = slice(c * APC, (c + 1) * APC)

        def pt(v):
            # t-major within-core layout: [p, t] holds anchor t*128 + p
            return np.ascontiguousarray(
                v[sl].reshape(T, P).T.astype(np.float32))

        im = dict(base)
        im["ax1"] = pt(x1)
        im["ay1"] = pt(y1)
        im["ax2p"] = pt(x2 + f1)
        im["ay2p"] = pt(y2 + f1)
        # outside anchors get a huge area => iou ~ 1e-5, never wins a per-gt
        # argmax (real per-gt maxima are >= ~0.015) and maps to label 0
        # pre-unmap, exactly as the reference's masked/unmapped semantics need
        im["areaa"] = pt(areaa + (~inside).astype(np.float32) * np.float32(1e9))
        im["ins01"] = pt(inside.astype(np.float32))
        e4 = np.stack([ecx[sl], ecy[sl], zeros, zeros], axis=1)
        im["e4"] = np.ascontiguousarray(
            e4.reshape(T, P, 4).transpose(1, 0, 2).reshape(P, 4 * T)
            .astype(np.float32))
        iew = np.float32(1.0) / ew[sl]
        ieh = np.float32(1.0) / eh[sl]
        ewh = np.stack([iew, ieh, iew, ieh], axis=1)
        im["ewh4"] = np.ascontiguousarray(
            ewh.reshape(T, P, 4).transpose(1, 0, 2).reshape(P, 4 * T)
            .astype(np.float32))
        insf = inside[sl].astype(np.float32)
        ins4 = np.stack([insf] * 4, axis=1)
        im["ins4"] = np.ascontiguousarray(
            ins4.reshape(T, P, 4).transpose(1, 0, 2).reshape(P, 4 * T)
            .astype(np.float32))
        in_maps.append(im)
    return in_maps, inside


def kernel(scores, gt_boxes, metadata):
    global LAST_RESULT
    in_maps, inside = _prepare(gt_boxes, metadata)
    nc = _get_graph()
    trace = bool(int(os.environ.get("BASSK_TRACE", "0")))
    res = run_bass_kernel_spmd(nc, in_maps, core_ids=list(range(N_CORES)),
                               trace=trace)
    LAST_RESULT = res
    results = res.results

    labels = np.empty(A, np.float32)
    bbox = np.empty((A, 4), np.float32)
    Mv = np.empty((N_CORES, G), np.float32)
    Iv = np.empty((N_CORES, G), np.int64)
    for c, r in enumerate(results):
        # [p, t] holds anchor t*128+p of this core's shard
        labels[c * APC:(c + 1) * APC] = np.asarray(r["labels"]).T.reshape(-1)
        bbox[c * APC:(c + 1) * APC] = (
            np.asarray(r["bbox"]).reshape(P, T, 4).transpose(1, 0, 2)
            .reshape(APC, 4))
        gm = np.asarray(r["gtmax"])
        gi = np.asarray(r["gtidx"])
        for ch in (0, 1):
            g = ch * P + np.arange(P)
            Mv[c, g] = gm[:, ch]
            # window position w*512 + j IS the within-core anchor index
            Iv[c, g] = c * APC + gi[:, ch].astype(np.int64)

    cstar = np.argmax(Mv, axis=0)
    gt_argmax = Iv[cstar, np.arange(G)]
    counts = np.bincount(gt_argmax, minlength=A)
    for a in np.nonzero(counts)[0]:
        if not inside[a]:
            continue
        k = counts[a]
        l = labels[a]
        if l == 0.0:
            labels[a] = np.float32(k)
        elif l == -1.0:
            labels[a] = np.float32(2 * k - 1)
    return labels[None, :], bbox[None, :, :]
